# revision 14
# baseline (speedup 1.0000x reference)
"""CenterLoss kernel for 8 Trainium2 NeuronCores.

The reference discards the addmm cross term, so

    loss = (1/B) * sum_i (||x_i||^2 + ||centers[y_i]||^2) + (C-1) * 1e-12

(the constant comes from clip(0, 1e-12, 1e12) applied to the B*(C-1) zero
entries of dist; the nonzero entries are ~1e2, far inside the clamp range).
The [B, C] matrix never needs to be materialized.

Data-parallel over batch: each core squares+row-sums its x slab (viewed as
[128, 2048]) and gathers its 2048 centers[y_i] rows with 16 indirect DMAs
(128 rows each, one row per partition; multi-offset-per-partition forms
mis-gather ~0.5% of rows on HW), squares+row-sums those, collapses the [128,1] partial to
one element with a PE matmul against ones (a [128,1]-strided output DMA
costs ~8us in completion latency; a 1-element DMA doesn't), and writes a
scalar. The host sums the 8 partials in float64. The gather wall is SWDGE
descriptor generation on the Q7 (~9ns/descriptor, ~19us for 2048 rows);
dma_gather batches descriptors but its mlp-library load gates the stream
~8us later, a wash -- 16 plain indirect DMAs start earlier and win.

Raw Bass (no Tile): this toolchain's walrus rejects Tile's tail drain and
runtime-faults on TENSOR_TENSOR_REDUCE, so sync is explicit. Extended
bass_isa instructions (dma_gather, load_library) need
library_overlay.lower_extended_insts(nc) or walrus sees empty .instr bytes
("ISA wrong length"). dma_gather needs gpsimd.load_library(mlp). DVE
same-engine RAW needs drain(). A semaphore may only be waited at its full
count (partial counts race: the 16 per-SDMA-engine increments of
concurrent DMAs interleave), hence one sem per DMA / gather chunk.
"""

from contextlib import ExitStack

import numpy as np

import concourse.bass as bass
from concourse import library_overlay, mybir
from concourse.bass import MemorySpace
from concourse.bass_utils import run_bass_kernel_spmd

B = 16384  # batch
D = 128  # feature dim
C = 10000  # num classes
M = 8  # cores
P = 128  # SBUF partitions
BPC = B // M  # 2048 batch rows per core
XFREE = BPC * D // P  # 2048 f32 per partition when x slab is viewed [P, -1]
NG = BPC // P  # 16 gathered-row groups of 128 (dest layout [P, NG, D])
NCHUNK = 8  # gather semaphore groups (full-count waits only)
GW = NG // NCHUNK

_F32 = mybir.dt.float32
_I32 = mybir.dt.int32


def _indirect_gather(gpsimd, out, in_, offset_ap, queue):
    """indirect_dma_start with an explicit SWDGE queue (alternating queues
    removes the inter-instruction ring backpressure gap)."""
    assert in_.space == MemorySpace.DRAM and out.space == MemorySpace.SBUF
    assert isinstance(in_.offset, int) and in_.offset == 0
    out_ap = gpsimd.lower_ap_dma(out, for_indirect_dma=True)
    in_ap = gpsimd.lower_ap_dma(in_, for_indirect_dma=True)
    off = gpsimd.lower_ap_dma(offset_ap)
    assert len(in_ap) == 1 and len(out_ap) == 1 and len(off) == 1
    in_ap.append(off[0])
    in_ap[0].dynamic_ap_info = mybir.DynamicAccessPatternInfo(
        c=0,
        actual_ap=out.ap,
        indirect_dim_max_index=in_.shape[0],
        offset_expr=[
            mybir.DynamicAccessPatternOffsetExpr(
                coef=in_.shape[1],
                aff_expr=mybir.DynamicAccessPatternOffsetExprAffExpr(
                    kind="IndirectArgId", arg_id=1
                ),
            )
        ],
    )
    return gpsimd.add_instruction(
        mybir.InstDMACopy(
            name=gpsimd.bass.get_next_instruction_name(),
            queue=queue,
            mode="Copy",
            ins=in_ap,
            outs=out_ap,
            oob_is_err=True,
            cce_op=mybir.AluOpType.bypass,
        )
    )


def _build_nc() -> bass.Bass:
    nc = bass.Bass(num_swdge_queues=2)
    xs = nc.dram_tensor("xs", [P, XFREE], _F32, kind="ExternalInput")
    ys = nc.dram_tensor("ys", [P, NG], _I32, kind="ExternalInput")
    cs = nc.dram_tensor("cs", [C, D], _F32, kind="ExternalInput")
    out = nc.dram_tensor("out", [1, 1], _F32, kind="ExternalOutput")

    with ExitStack() as ctx:
        e = ctx.enter_context
        y_sb = e(nc.sbuf_tensor([P, NG], _I32))
        warm_off = e(nc.sbuf_tensor([P, 1], _I32))
        warm_dst = e(nc.sbuf_tensor([P, D], _F32))
        g_sb = e(nc.sbuf_tensor([P, NG, D], _F32))
        x_sb = e(nc.sbuf_tensor([P, XFREE], _F32))
        tr_sb = e(nc.sbuf_tensor([P, XFREE], _F32))
        trg_sb = e(nc.sbuf_tensor([P, NG, D], _F32))
        acc_sb = e(nc.sbuf_tensor([P, 10], _F32))
        ones_sb = e(nc.sbuf_tensor([P, 1], _F32))
        fin_sb = e(nc.sbuf_tensor([1, 1], _F32))
        ps = e(nc.psum_tensor([1, 1], _F32))
        wsem = e(nc.semaphore())
        ysem = e(nc.semaphore())
        xsem = e(nc.semaphore())
        gsems = [e(nc.semaphore(f"gsem{i}")) for i in range(NCHUNK)]
        vsem = e(nc.semaphore())
        tsem = e(nc.semaphore())
        csem = e(nc.semaphore())
        osem = e(nc.semaphore())
        block = e(nc.Block())

        @block.sync
        def _(sync):
            sync.dma_start(out=y_sb[:], in_=ys[:]).then_inc(ysem, 16)
            sync.dma_start(out=x_sb[:], in_=xs[:]).then_inc(xsem, 16)
            sync.wait_ge(csem, 1)
            sync.dma_start(out=out[:], in_=fin_sb[:]).then_inc(osem, 16)
            sync.wait_ge(osem, 16)

        # group boundaries: front-load so the tail group is a single gather
        bounds = [0, 3, 5, 7, 9, 11, 13, 15, NG]

        @block.gpsimd
        def _(gpsimd):
            # warm the SWDGE indirect path while the y DMA is in flight
            gpsimd.memset(warm_off[:], 0)
            gpsimd.drain()
            _indirect_gather(
                gpsimd, warm_dst[:], cs[:], warm_off[:], queue="qPoolDynamic"
            ).then_inc(wsem, 16)
            gpsimd.wait_ge(wsem, 16)
            gpsimd.wait_ge(ysem, 16)
            for g in range(NG):
                grp = next(k for k in range(NCHUNK) if bounds[k] <= g < bounds[k + 1])
                _indirect_gather(
                    gpsimd,
                    g_sb[:, g, :],
                    cs[:],
                    y_sb[:, g : g + 1],
                    queue=f"qPoolDynamic{'' if g % 2 == 0 else '1'}",
                ).then_inc(gsems[grp], 16)

        @block.vector
        def _(vector):
            vector.memset(ones_sb[:], 1.0)
            vector.wait_ge(xsem, 16)
            vector.tensor_mul(tr_sb[:], x_sb[:], x_sb[:])
            for k in range(NCHUNK):
                vector.wait_ge(gsems[k], 16 * (bounds[k + 1] - bounds[k]))
                sl = slice(bounds[k], bounds[k + 1])
                gf = g_sb[:, sl, :].rearrange("p a b -> p (a b)")
                tf = trg_sb[:, sl, :].rearrange("p a b -> p (a b)")
                vector.tensor_mul(tf, gf, gf)
                vector.drain()
                if k == 0:
                    vector.reduce_sum(
                        acc_sb[:, 0:1], tr_sb[:], axis=mybir.AxisListType.X
                    )
                vector.reduce_sum(
                    acc_sb[:, 1 + k : 2 + k], tf, axis=mybir.AxisListType.X
                )
            vector.drain()
            vector.reduce_sum(acc_sb[:, 9:10], acc_sb[:, 0:9], axis=mybir.AxisListType.X)
            vector.drain()
            vector.nop().then_inc(vsem, 1)
            vector.wait_ge(tsem, 1)
            vector.tensor_copy(fin_sb[:], ps[:])
            vector.drain()
            vector.nop().then_inc(csem, 1)

        @block.tensor
        def _(tensor):
            tensor.wait_ge(vsem, 1)
            nc.tensor.matmul(
                ps[:], lhsT=acc_sb[:, 9:10], rhs=ones_sb[:], start=True, stop=True
            ).then_inc(tsem, 1)

    library_overlay.lower_extended_insts(nc)
    return nc


_NC_CACHE: list = []


def _get_nc() -> bass.Bass:
    if not _NC_CACHE:
        _NC_CACHE.append(_build_nc())
    return _NC_CACHE[0]


def _in_maps(x: np.ndarray, centers: np.ndarray, y: np.ndarray) -> list[dict]:
    x = np.ascontiguousarray(np.asarray(x, dtype=np.float32))
    centers = np.ascontiguousarray(np.asarray(centers, dtype=np.float32))
    y64 = np.asarray(y).reshape(B)
    maps = []
    for k in range(M):
        maps.append(
            {
                "xs": x[k * BPC : (k + 1) * BPC].reshape(P, XFREE),
                "ys": np.ascontiguousarray(
                    y64[k * BPC : (k + 1) * BPC].astype(np.int32).reshape(P, NG)
                ),
                "cs": centers,
            }
        )
    return maps


def _finalize(results: list[dict]) -> np.ndarray:
    total = 0.0
    for r in results:
        total += float(np.sum(r["out"].astype(np.float64)))
    loss = total / B + (C - 1) * 1e-12
    return np.float32(loss)


def run(x, centers, y, **spmd_kwargs):
    """Run on 8 cores; returns (loss, BassKernelResults)."""
    nc = _get_nc()
    res = run_bass_kernel_spmd(nc, _in_maps(x, centers, y), list(range(M)), **spmd_kwargs)
    return _finalize(res.results), res


def kernel(x: np.ndarray, centers: np.ndarray, y: np.ndarray) -> np.ndarray:
    loss, _ = run(x, centers, y)
    return loss


# revision 15
# speedup vs baseline: 1.0975x; 1.0975x over previous
"""CenterLoss kernel for 8 Trainium2 NeuronCores.

The reference discards the addmm cross term, so

    loss = (1/B) * sum_i (||x_i||^2 + ||centers[y_i]||^2) + (C-1) * 1e-12

(the constant comes from clip(0, 1e-12, 1e12) applied to the B*(C-1) zero
entries of dist; the nonzero entries are ~1e2, far inside the clamp range).
The [B, C] matrix never needs to be materialized.

Data-parallel over batch: each core squares+row-sums its x slab (viewed as
[128, 2048]) and gathers its 2048 centers[y_i] rows with 16 indirect DMAs
(128 rows each, one row per partition; multi-offset-per-partition forms
mis-gather ~0.5% of rows on HW), squares+row-sums those, collapses the [128,1] partial to
one element with a PE matmul against ones (a [128,1]-strided output DMA
costs ~8us in completion latency; a 1-element DMA doesn't), and writes a
scalar. The host sums the 8 partials in float64. The gather wall is SWDGE
descriptor generation on the Q7 (~9ns/descriptor, ~19us for 2048 rows);
dma_gather batches descriptors but its mlp-library load gates the stream
~8us later, a wash -- 16 plain indirect DMAs start earlier and win.

Raw Bass (no Tile): this toolchain's walrus rejects Tile's tail drain and
runtime-faults on TENSOR_TENSOR_REDUCE, so sync is explicit. Extended
bass_isa instructions (dma_gather, load_library) need
library_overlay.lower_extended_insts(nc) or walrus sees empty .instr bytes
("ISA wrong length"). dma_gather needs gpsimd.load_library(mlp). DVE
same-engine RAW needs drain(). A semaphore may only be waited at its full
count (partial counts race: the 16 per-SDMA-engine increments of
concurrent DMAs interleave), hence one sem per DMA / gather chunk.
"""

from contextlib import ExitStack

import numpy as np

import concourse.bass as bass
from concourse import library_overlay, mybir
from concourse.bass import MemorySpace
from concourse.bass_utils import run_bass_kernel_spmd

B = 16384  # batch
D = 128  # feature dim
C = 10000  # num classes
M = 8  # cores
P = 128  # SBUF partitions
BPC = B // M  # 2048 batch rows per core
XFREE = BPC * D // P  # 2048 f32 per partition when x slab is viewed [P, -1]
NG = BPC // P  # 16 gathered-row groups of 128 (dest layout [P, NG, D])
NCHUNK = 8  # gather semaphore groups (full-count waits only)
GW = NG // NCHUNK

_F32 = mybir.dt.float32
_I32 = mybir.dt.int32


def _indirect_gather(gpsimd, out, in_, offset_ap, queue):
    """indirect_dma_start with an explicit SWDGE queue (alternating queues
    removes the inter-instruction ring backpressure gap)."""
    assert in_.space == MemorySpace.DRAM and out.space == MemorySpace.SBUF
    assert isinstance(in_.offset, int) and in_.offset == 0
    out_ap = gpsimd.lower_ap_dma(out, for_indirect_dma=True)
    in_ap = gpsimd.lower_ap_dma(in_, for_indirect_dma=True)
    off = gpsimd.lower_ap_dma(offset_ap)
    assert len(in_ap) == 1 and len(out_ap) == 1 and len(off) == 1
    in_ap.append(off[0])
    in_ap[0].dynamic_ap_info = mybir.DynamicAccessPatternInfo(
        c=0,
        actual_ap=out.ap,
        indirect_dim_max_index=in_.shape[0],
        offset_expr=[
            mybir.DynamicAccessPatternOffsetExpr(
                coef=in_.shape[1],
                aff_expr=mybir.DynamicAccessPatternOffsetExprAffExpr(
                    kind="IndirectArgId", arg_id=1
                ),
            )
        ],
    )
    return gpsimd.add_instruction(
        mybir.InstDMACopy(
            name=gpsimd.bass.get_next_instruction_name(),
            queue=queue,
            mode="Copy",
            ins=in_ap,
            outs=out_ap,
            oob_is_err=True,
            cce_op=mybir.AluOpType.bypass,
        )
    )


def _build_nc() -> bass.Bass:
    nc = bass.Bass(num_swdge_queues=2)
    xs = nc.dram_tensor("xs", [P, XFREE], _F32, kind="ExternalInput")
    ys = nc.dram_tensor("ys", [P, NG], _I32, kind="ExternalInput")
    cs = nc.dram_tensor("cs", [C, D], _F32, kind="ExternalInput")
    out = nc.dram_tensor("out", [1, 1], _F32, kind="ExternalOutput")

    with ExitStack() as ctx:
        e = ctx.enter_context
        y_sb = e(nc.sbuf_tensor([P, NG], _I32))
        g_sb = e(nc.sbuf_tensor([P, NG, D], _F32))
        x_sb = e(nc.sbuf_tensor([P, XFREE], _F32))
        tr_sb = e(nc.sbuf_tensor([P, XFREE], _F32))
        trg_sb = e(nc.sbuf_tensor([P, NG, D], _F32))
        acc_sb = e(nc.sbuf_tensor([P, 10], _F32))
        ones_sb = e(nc.sbuf_tensor([P, 1], _F32))
        fin_sb = e(nc.sbuf_tensor([1, 1], _F32))
        ps = e(nc.psum_tensor([1, 1], _F32))
        ysem = e(nc.semaphore())
        xsem = e(nc.semaphore())
        gsems = [e(nc.semaphore(f"gsem{i}")) for i in range(NCHUNK)]
        vsem = e(nc.semaphore())
        tsem = e(nc.semaphore())
        csem = e(nc.semaphore())
        osem = e(nc.semaphore())
        block = e(nc.Block())

        @block.sync
        def _(sync):
            sync.dma_start(out=y_sb[:], in_=ys[:]).then_inc(ysem, 16)
            sync.dma_start(out=x_sb[:], in_=xs[:]).then_inc(xsem, 16)
            sync.wait_ge(csem, 1)
            sync.dma_start(out=out[:], in_=fin_sb[:]).then_inc(osem, 16)
            sync.wait_ge(osem, 16)

        @block.gpsimd
        def _(gpsimd):
            gpsimd.wait_ge(ysem, 16)
            for g in range(NG):
                _indirect_gather(
                    gpsimd,
                    g_sb[:, g, :],
                    cs[:],
                    y_sb[:, g : g + 1],
                    queue=f"qPoolDynamic{'' if g % 2 == 0 else '1'}",
                ).then_inc(gsems[g // GW], 16)

        @block.vector
        def _(vector):
            vector.memset(ones_sb[:], 1.0)
            vector.wait_ge(xsem, 16)
            vector.tensor_mul(tr_sb[:], x_sb[:], x_sb[:])
            for k in range(NCHUNK):
                vector.wait_ge(gsems[k], 16 * GW)
                sl = slice(k * GW, (k + 1) * GW)
                gf = g_sb[:, sl, :].rearrange("p a b -> p (a b)")
                tf = trg_sb[:, sl, :].rearrange("p a b -> p (a b)")
                vector.tensor_mul(tf, gf, gf)
                vector.drain()
                if k == 0:
                    vector.reduce_sum(
                        acc_sb[:, 0:1], tr_sb[:], axis=mybir.AxisListType.X
                    )
                vector.reduce_sum(
                    acc_sb[:, 1 + k : 2 + k], tf, axis=mybir.AxisListType.X
                )
            vector.drain()
            vector.reduce_sum(acc_sb[:, 9:10], acc_sb[:, 0:9], axis=mybir.AxisListType.X)
            vector.drain()
            vector.nop().then_inc(vsem, 1)
            vector.wait_ge(tsem, 1)
            vector.tensor_copy(fin_sb[:], ps[:])
            vector.drain()
            vector.nop().then_inc(csem, 1)

        @block.tensor
        def _(tensor):
            tensor.wait_ge(vsem, 1)
            nc.tensor.matmul(
                ps[:], lhsT=acc_sb[:, 9:10], rhs=ones_sb[:], start=True, stop=True
            ).then_inc(tsem, 1)

    library_overlay.lower_extended_insts(nc)
    return nc


_NC_CACHE: list = []


def _get_nc() -> bass.Bass:
    if not _NC_CACHE:
        _NC_CACHE.append(_build_nc())
    return _NC_CACHE[0]


def _in_maps(x: np.ndarray, centers: np.ndarray, y: np.ndarray) -> list[dict]:
    x = np.ascontiguousarray(np.asarray(x, dtype=np.float32))
    centers = np.ascontiguousarray(np.asarray(centers, dtype=np.float32))
    y64 = np.asarray(y).reshape(B)
    maps = []
    for k in range(M):
        maps.append(
            {
                "xs": x[k * BPC : (k + 1) * BPC].reshape(P, XFREE),
                "ys": np.ascontiguousarray(
                    y64[k * BPC : (k + 1) * BPC].astype(np.int32).reshape(P, NG)
                ),
                "cs": centers,
            }
        )
    return maps


def _finalize(results: list[dict]) -> np.ndarray:
    total = 0.0
    for r in results:
        total += float(np.sum(r["out"].astype(np.float64)))
    loss = total / B + (C - 1) * 1e-12
    return np.float32(loss)


def run(x, centers, y, **spmd_kwargs):
    """Run on 8 cores; returns (loss, BassKernelResults)."""
    nc = _get_nc()
    res = run_bass_kernel_spmd(nc, _in_maps(x, centers, y), list(range(M)), **spmd_kwargs)
    return _finalize(res.results), res


def kernel(x: np.ndarray, centers: np.ndarray, y: np.ndarray) -> np.ndarray:
    loss, _ = run(x, centers, y)
    return loss
